# revision 28
# baseline (speedup 1.0000x reference)
"""Trainium2 Bass kernel for nn_BatchTreeEncoder (batched tree-GRU encoder).

Strategy
--------
Pure data parallel over the batch: 256 trees -> 32 trees on each of the 8
NeuronCores, weights replicated.  Inside one core all activations are kept in
a transposed [E, nodes] layout (E-chunks of 128 on partitions, nodes on the
free dim), so the only on-chip transposes are of the gathered embeddings.

Per level (bottom-up), in node-chunks:
  - indirect-DMA gather of embedding rows (fp8 table, x16 scale), PE-transpose
    to X^T in fp8
  - GRU gates as fp8 DoubleRow matmuls accumulated in f32 PSUM (weights x64,
    activations x16; the 1/1024 is folded into the activation scale); GX and
    GH for the r/z gates accumulate into the *same* PSUM bank, the n gate
    keeps them separate because of the `r *` coupling
  - sigmoid is computed as 0.5*(1+tanh(x/2)) so that tanh/exp are the only
    ScalarE table functions used (single ACT table set, no reloads)
  - child attention for the parent level is fused right after a chunk's H is
    produced: U = tanh(Ws^T H), s = tanh(cw^T U), 3-way softmax on a [1, N]
    row, weights broadcast to 128 partitions via a K=1 matmul, weighted sum
    by grouped strided adds
  - running per-tree elementwise max is folded in as each H chunk completes
Output: PE-transpose of the [E, 32] max back to [32, E] and DMA out.
"""

import sys

for _p in ("/opt/trn_rl_repo",):
    if _p not in sys.path:
        sys.path.insert(0, _p)

import numpy as np
import ml_dtypes

bf16 = ml_dtypes.bfloat16

# ---------------------------------------------------------------- constants
NCORES = 8
BS = 256
T = BS // NCORES          # trees per core
K = 3
DEPTH = 4
E = 1024
EC = E // 128             # 8 e-chunks
VOCAB = 20000
N_NODES = sum(K**l for l in range(DEPTH + 1))   # 121
# fp8 quantization scales: weights x64, activations/emb x16 (keeps the bulk of
# the values out of e4m3's subnormal range); PSUM results carry x1024, undone
# in the activation `scale` params.
SW = 64.0
SX = 16.0
SCL = SW * SX
LEVEL_OFF = [sum(K**i for i in range(l)) for l in range(DEPTH + 1)]  # [0,1,4,13,40]

# node-chunk sizes per level; every chunk size is a multiple of 3^l (whole
# trees stay inside one chunk for the max-reduction) and of 3 for l>0 (whole
# sibling groups stay inside one chunk for the parent attention).
CHUNK_SIZES = {4: [486] * 5 + [162], 3: [432] * 2, 2: [288], 1: [96], 0: [32]}


def _schedule():
    """Static per-core schedule: list of (level, c0, nc, [(gblock, boff, rows)])."""
    sched = []
    gb = 0
    for l in range(DEPTH, -1, -1):
        c0 = 0
        for nc_ in CHUNK_SIZES[l]:
            blocks = []
            boff = 0
            while boff < nc_:
                rows = min(128, nc_ - boff)
                blocks.append((gb, boff, rows))
                gb += 1
                boff += rows
            sched.append((l, c0, nc_, blocks))
            c0 += nc_
    return sched, gb


SCHEDULE, NB = _schedule()

_NC_CACHE = {}


# ---------------------------------------------------------------- builder
def build_nc():
    import concourse.bacc as bacc
    import concourse.bass as bass
    import concourse.mybir as mybir
    import concourse.tile as tile
    from concourse.masks import make_identity

    dt = mybir.dt
    Act = mybir.ActivationFunctionType
    Alu = mybir.AluOpType
    X = mybir.AxisListType.X

    nc = bacc.Bacc("TRN2", target_bir_lowering=False, debug=False)

    tok_d = nc.dram_tensor("tok", [NB, 128], dt.int32, kind="ExternalInput")
    emb_d = nc.dram_tensor("emb", [VOCAB, E], dt.bfloat16, kind="ExternalInput")
    wiT_d = nc.dram_tensor("wiT", [3 * EC, EC // 2, 128, 2, 128], dt.float8e4, kind="ExternalInput")
    whT_d = nc.dram_tensor("whT", [3 * EC, EC // 2, 128, 2, 128], dt.float8e4, kind="ExternalInput")
    ws_d = nc.dram_tensor("ws", [EC, EC // 2, 128, 2, 128], dt.float8e4, kind="ExternalInput")
    ctx_d = nc.dram_tensor("ctxw", [EC, 128, 1], dt.bfloat16, kind="ExternalInput")
    bias_d = nc.dram_tensor("bias", [128, 56], dt.float32, kind="ExternalInput")
    out_d = nc.dram_tensor("out", [T, E], dt.float32, kind="ExternalOutput")

    from contextlib import ExitStack

    with tile.TileContext(nc) as tc, ExitStack() as ctx:
        sing = ctx.enter_context(tc.tile_pool(name="sing", bufs=1))
        hsp = ctx.enter_context(tc.tile_pool(name="hsp", bufs=1))
        mp_ = ctx.enter_context(tc.tile_pool(name="mp", bufs=1))
        xrowp = ctx.enter_context(tc.tile_pool(name="xrowp", bufs=4))
        xtp = ctx.enter_context(tc.tile_pool(name="xtp", bufs=3))
        gp = ctx.enter_context(tc.tile_pool(name="gp", bufs=2))      # r/z/n/t tiles
        hp = ctx.enter_context(tc.tile_pool(name="hp", bufs=2))      # H chunks
        up = ctx.enter_context(tc.tile_pool(name="up", bufs=2))      # U tiles
        rowp = ctx.enter_context(tc.tile_pool(name="rowp", bufs=1))  # softmax rows
        wp = ctx.enter_context(tc.tile_pool(name="wp", bufs=2))      # bcast weights etc
        psp = ctx.enter_context(tc.tile_pool(name="psp", bufs=1, space="PSUM"))

        # ---- persistent / constant tiles
        wiT = sing.tile([128, 3 * EC, EC // 2, 2, 128], dt.float8e4)
        whT = sing.tile([128, 3 * EC, EC // 2, 2, 128], dt.float8e4)
        ws = sing.tile([128, EC, EC // 2, 2, 128], dt.float8e4)
        ctxw = sing.tile([128, EC, 1], dt.bfloat16)
        biases = sing.tile([128, 56], dt.float32)
        identb = sing.tile([128, 128], dt.bfloat16)
        identf = sing.tile([128, 128], dt.float32)
        ones = sing.tile([1, 128], dt.bfloat16)
        idx = sing.tile([128, NB], dt.int32)

        # index + small tensors first (they gate the gather -> transpose chain),
        # then weights gate-chunk by gate-chunk in consumption order: Wi fully
        # before Wh (Wh is first needed only at level 3, ~350us in).
        nc.sync.dma_start(out=idx[:], in_=tok_d.rearrange("b p -> p b"))
        nc.sync.dma_start(out=biases[:], in_=bias_d[:])
        nc.sync.dma_start(out=ctxw[:, :, 0], in_=ctx_d.rearrange("k p o -> p (k o)"))
        make_identity(nc, identb[:])
        make_identity(nc, identf[:])
        nc.vector.memset(ones[:], 1.0)
        # weight streams: issue in per-e consumption order (r_e, z_e, n_e),
        # round-robined over three DMA queues so the leaf level's first chunks
        # never wait on a serialized weight stream.
        gorder = [base + e for e in range(EC) for base in (0, EC, 2 * EC)]
        for g in gorder:
            nc.sync.dma_start(out=wiT[:, g, :, :, :],
                              in_=wiT_d[g].rearrange("j p i m -> p j i m"))
        for g in range(EC):
            nc.sync.dma_start(out=ws[:, g, :, :, :],
                              in_=ws_d[g].rearrange("j p i m -> p j i m"))
        for g in gorder:
            nc.sync.dma_start(out=whT[:, g, :, :, :],
                              in_=whT_d[g].rearrange("j p i m -> p j i m"))

        # bias column helpers: cols 0..7 = 0.5*(bi+bh)_r, 8..15 = 0.5*(bi+bh)_z,
        # 16..23 = bi_n, 24..31 = SCL*bh_n, 32..39 = sent_bias,
        # 40..47 = 0.5*SCL*bh_n (leaf), 48..55 = bi_n + 0.5*bh_n (leaf)
        def bcol(c):
            return biases[:, c:c + 1]

        # running max, [128, EC, T] f32
        msb = mp_.tile([128, EC, T], dt.float32)
        nc.vector.memset(msb[:], -3.0e38)

        # per-level HS accumulation targets ([E, N_l] as [128, EC, N_l] bf16)
        hs_sb = {}
        hs8_sb = {}
        for l in range(DEPTH):
            n_l = T * K**l
            hs_sb[l] = hsp.tile([128, EC, n_l], dt.bfloat16, name=f"hs{l}")
            hs8_sb[l] = hsp.tile([128, EC, n_l], dt.float8e4, name=f"hs8{l}")

        DR = mybir.MatmulPerfMode.DoubleRow

        def gh_mms(out_ap, g, lvl, c0, ncn, start):
            src8 = hs8_sb[lvl]
            if ncn >= 128:
                for j in range(EC // 2):
                    nc.tensor.matmul(
                        out=out_ap, lhsT=whT[:, g, j, :, :],
                        rhs=src8[:, 2 * j:2 * j + 2, c0:c0 + ncn],
                        start=(start and j == 0), stop=(j == EC // 2 - 1),
                        perf_mode=DR)
            else:
                for k in range(EC):
                    nc.tensor.matmul(
                        out=out_ap, lhsT=whT[:, g, k // 2, k % 2, :],
                        rhs=src8[:, k, c0:c0 + ncn],
                        start=(start and k == 0), stop=(k == EC - 1))

        def gx_mms(out_ap, g, xt, ncn, stop):
            if ncn >= 128:
                for j in range(EC // 2):
                    nc.tensor.matmul(
                        out=out_ap, lhsT=wiT[:, g, j, :, :],
                        rhs=xt[:, 2 * j:2 * j + 2, :ncn],
                        start=(j == 0), stop=(stop and j == EC // 2 - 1),
                        perf_mode=DR)
            else:
                for k in range(EC):
                    nc.tensor.matmul(
                        out=out_ap, lhsT=wiT[:, g, k // 2, k % 2, :],
                        rhs=xt[:, k, :ncn],
                        start=(k == 0), stop=(stop and k == EC - 1))

        def u_mms(out_ap, f, h8, ncn):
            if ncn >= 128:
                for j in range(EC // 2):
                    nc.tensor.matmul(
                        out=out_ap, lhsT=ws[:, f, j, :, :],
                        rhs=h8[:, 2 * j:2 * j + 2, :],
                        start=(j == 0), stop=(j == EC // 2 - 1), perf_mode=DR)
            else:
                for k in range(EC):
                    nc.tensor.matmul(
                        out=out_ap, lhsT=ws[:, f, k // 2, k % 2, :],
                        rhs=h8[:, k, :],
                        start=(k == 0), stop=(k == EC - 1))

        def emit_xt(lvl, c0, ncn, blocks):
            # gather + transpose (bf16) -> xt [128, EC, ncn] fp8 (x16-scaled,
            # quantization fused into the PSUM->SBUF copy)
            xrows = []
            for (gb, boff, rows) in blocks:
                xrow = xrowp.tile([128, E], dt.bfloat16, name="xrow")
                nc.gpsimd.indirect_dma_start(
                    out=xrow[:rows, :],
                    out_offset=None,
                    in_=emb_d[:, :],
                    in_offset=bass.IndirectOffsetOnAxis(ap=idx[:rows, gb:gb + 1], axis=0),
                )
                xrows.append((xrow, boff, rows))
            xt = xtp.tile([128, EC, ncn], dt.float8e4, name="xt")
            for e in range(EC):
                tp = psp.tile([128, 512], dt.bfloat16, name="tp", tag="tp", bufs=2)
                for (xrow, boff, rows) in xrows:
                    nc.tensor.transpose(
                        out=tp[:, boff:boff + rows],
                        in_=xrow[:rows, e * 128:(e + 1) * 128],
                        identity=identb[:rows, :rows],
                    )
                nc.scalar.mul(out=xt[:, e, :], in_=tp[:, :ncn], mul=SX)
            return xt

        xts = {0: emit_xt(*[SCHEDULE[0][i] for i in range(4)])}
        for ci, (lvl, c0, ncn, blocks) in enumerate(SCHEDULE):
            leaf = lvl == DEPTH
            n_per_tree = K**lvl
            tr0 = c0 // n_per_tree
            ntr = ncn // n_per_tree
            if ci + 1 < len(SCHEDULE):
                xts[ci + 1] = emit_xt(*[SCHEDULE[ci + 1][i] for i in range(4)])
            xt = xts.pop(ci)

            # ---------------- GRU (per e-chunk)
            hch = hp.tile([128, EC, ncn], dt.bfloat16, name="hch")
            hch8 = None
            if lvl > 0:
                hch8 = hp.tile([128, EC, ncn], dt.float8e4, name="hch8", tag="hch8")
            for e in range(EC):
                # r gate
                psr = psp.tile([128, 512], dt.float32, name="psr", tag="acc", bufs=2)
                gx_mms(psr[:, :ncn], e, xt, ncn, stop=leaf)
                if not leaf:
                    gh_mms(psr[:, :ncn], e, lvl, c0, ncn, start=False)
                # r tile holds rt = tanh(rin/2); non-leaf converts to
                # r = 0.5 + 0.5*rt, the leaf path folds that into the n-gate
                # bias columns instead.
                r = gp.tile([128, 512], dt.bfloat16, name="r", tag="r")
                nc.scalar.activation(r[:, :ncn], psr[:, :ncn], Act.Tanh,
                                     bias=bcol(e), scale=0.5 / SCL)
                if not leaf:
                    nc.vector.tensor_scalar(r[:, :ncn], r[:, :ncn], 0.5, 0.5,
                                            Alu.mult, Alu.add)
                # z gate (kept as zt = tanh(zin/2))
                psz = psp.tile([128, 512], dt.float32, name="psz", tag="acc", bufs=2)
                gx_mms(psz[:, :ncn], EC + e, xt, ncn, stop=leaf)
                if not leaf:
                    gh_mms(psz[:, :ncn], EC + e, lvl, c0, ncn, start=False)
                zt = gp.tile([128, 512], dt.bfloat16, name="zt", tag="zt")
                nc.scalar.activation(zt[:, :ncn], psz[:, :ncn], Act.Tanh,
                                     bias=bcol(8 + e), scale=0.5 / SCL)
                # n gate
                psx = psp.tile([128, 512], dt.float32, name="psx", tag="gxn", bufs=1)
                gx_mms(psx[:, :ncn], 2 * EC + e, xt, ncn, stop=True)
                tt = gp.tile([128, 512], dt.bfloat16, name="tt", tag="tt")
                if leaf:
                    # tt = rt * 0.5*bh_n + GXn  (r = 0.5+0.5*rt fold; the
                    # remaining 0.5*bh_n constant lives in the bias column)
                    nc.vector.scalar_tensor_tensor(
                        out=tt[:, :ncn], in0=r[:, :ncn], scalar=bcol(40 + e),
                        in1=psx[:, :ncn], op0=Alu.mult, op1=Alu.add)
                else:
                    psh = psp.tile([128, 512], dt.float32, name="psh", tag="ghn", bufs=2)
                    gh_mms(psh[:, :ncn], 2 * EC + e, lvl, c0, ncn, start=True)
                    # tt = (GHn + bh_n) * r ; then += GXn
                    nc.vector.scalar_tensor_tensor(
                        out=tt[:, :ncn], in0=psh[:, :ncn], scalar=bcol(24 + e),
                        in1=r[:, :ncn], op0=Alu.add, op1=Alu.mult)
                    nc.vector.tensor_add(tt[:, :ncn], tt[:, :ncn], psx[:, :ncn])
                n = gp.tile([128, 512], dt.bfloat16, name="n", tag="n")
                nc.scalar.activation(n[:, :ncn], tt[:, :ncn], Act.Tanh,
                                     bias=bcol((48 if leaf else 16) + e),
                                     scale=1.0 / SCL)
                # blend -> H
                if leaf:
                    nc.vector.tensor_scalar(zt[:, :ncn], zt[:, :ncn], -0.5, 0.5,
                                            Alu.mult, Alu.add)
                    nc.vector.tensor_mul(hch[:, e, :], zt[:, :ncn], n[:, :ncn])
                else:
                    dd = gp.tile([128, 512], dt.bfloat16, name="dd", tag="dd")
                    nc.vector.tensor_sub(dd[:, :ncn], hs_sb[lvl][:, e, c0:c0 + ncn],
                                         n[:, :ncn])
                    mm_ = gp.tile([128, 512], dt.bfloat16, name="mm_", tag="mm_")
                    nc.vector.tensor_mul(mm_[:, :ncn], zt[:, :ncn], dd[:, :ncn])
                    nc.vector.tensor_add(mm_[:, :ncn], dd[:, :ncn], mm_[:, :ncn])
                    nc.vector.scalar_tensor_tensor(
                        out=hch[:, e, :], in0=mm_[:, :ncn], scalar=0.5,
                        in1=n[:, :ncn], op0=Alu.mult, op1=Alu.add)
                if hch8 is not None:
                    nc.gpsimd.tensor_scalar_mul(out=hch8[:, e, :],
                                                in0=hch[:, e, :], scalar1=SX)
                # running max for this e-chunk
                if n_per_tree == 1:
                    nc.vector.tensor_max(msb[:, e, tr0:tr0 + ntr],
                                         msb[:, e, tr0:tr0 + ntr], hch[:, e, :])
                else:
                    red = wp.tile([128, T], dt.float32, name="red", tag="red")
                    nc.vector.reduce_max(
                        out=red[:, :ntr],
                        in_=hch[:, e, :].rearrange("p (t n) -> p t n", n=n_per_tree),
                        axis=X)
                    nc.vector.tensor_max(msb[:, e, tr0:tr0 + ntr],
                                         msb[:, e, tr0:tr0 + ntr], red[:, :ntr])

            # ---------------- fused attention for the parent level
            if lvl > 0:
                npar = ncn // 3
                p0 = c0 // 3
                lp = lvl - 1
                pss = psp.tile([1, 512], dt.float32, name="pss", tag="srow", bufs=1)
                for f in range(EC):
                    psu = psp.tile([128, 512], dt.float32, name="psu", tag="acc", bufs=2)
                    u_mms(psu[:, :ncn], f, hch8, ncn)
                    ut = up.tile([128, 512], dt.bfloat16, name="ut", tag="ut")
                    nc.scalar.activation(ut[:, :ncn], psu[:, :ncn], Act.Tanh,
                                         bias=bcol(32 + f), scale=1.0 / SCL)
                    nc.tensor.matmul(out=pss[:, :ncn], lhsT=ctxw[:, f, 0:1],
                                     rhs=ut[:, :ncn],
                                     start=(f == 0), stop=(f == EC - 1))
                # softmax: tanh -> broadcast to 128 partitions via ones-matmul
                # -> wide exp / 3-way normalize (no 1-partition DVE ops)
                srow = rowp.tile([1, 512], dt.bfloat16, name="srow", tag="srow")
                nc.scalar.activation(srow[:, :ncn], pss[:, :ncn], Act.Tanh)
                psb = psp.tile([128, 512], dt.float32, name="psb", tag="ghn", bufs=2)
                nc.tensor.matmul(out=psb[:, :ncn], lhsT=ones[:, :],
                                 rhs=srow[:, :ncn], start=True, stop=True)
                ebc = wp.tile([128, 512], dt.bfloat16, name="ebc", tag="ebc")
                nc.scalar.activation(ebc[:, :ncn], psb[:, :ncn], Act.Exp)
                e3 = ebc[:, :ncn].rearrange("p (n k) -> p n k", k=3)
                den = wp.tile([128, 170], dt.float32, name="den", tag="den")
                nc.vector.tensor_add(den[:, :npar], e3[:, :, 0], e3[:, :, 1])
                nc.vector.tensor_add(den[:, :npar], den[:, :npar], e3[:, :, 2])
                rinv = wp.tile([128, 170], dt.float32, name="rinv", tag="rinv")
                nc.vector.reciprocal(rinv[:, :npar], den[:, :npar])
                wb = wp.tile([128, 512], dt.bfloat16, name="wb", tag="wb")
                w3 = wb[:, :ncn].rearrange("p (n k) -> p n k", k=3)
                for kk in range(3):
                    nc.vector.tensor_mul(w3[:, :, kk], e3[:, :, kk], rinv[:, :npar])
                for e in range(EC):
                    pp = wp.tile([128, 512], dt.bfloat16, name="pp", tag="pp")
                    nc.vector.tensor_mul(pp[:, :ncn], hch[:, e, :], wb[:, :ncn])
                    p3 = pp[:, :ncn].rearrange("p (n k) -> p n k", k=3)
                    ta = wp.tile([128, 170], dt.bfloat16, name="ta", tag="ta")
                    nc.vector.tensor_add(ta[:, :npar], p3[:, :, 0], p3[:, :, 1])
                    nc.vector.tensor_add(hs_sb[lp][:, e, p0:p0 + npar],
                                         ta[:, :npar], p3[:, :, 2])
                    nc.gpsimd.tensor_scalar_mul(out=hs8_sb[lp][:, e, p0:p0 + npar],
                                                in0=hs_sb[lp][:, e, p0:p0 + npar],
                                                scalar1=SX)

        # ---------------- output: transpose msb -> [T, E], DMA out
        osb = mp_.tile([T, E], dt.float32)
        for e in range(EC):
            po = psp.tile([T, 128], dt.float32, name="po", tag="tp", bufs=2)
            nc.tensor.transpose(out=po[:, :], in_=msb[:, e, :], identity=identf[:, :])
            nc.vector.tensor_copy(out=osb[:, e * 128:(e + 1) * 128], in_=po[:, :])
        nc.sync.dma_start(out=out_d[:, :], in_=osb[:, :])

    nc.compile()
    return nc


def get_nc():
    if "nc" not in _NC_CACHE:
        _NC_CACHE["nc"] = build_nc()
    return _NC_CACHE["nc"]


# ---------------------------------------------------------------- host side
def _prep_shared(emb, gru_Wi, gru_Wh, gru_bi, gru_bh, sent_weight, sent_bias,
                 context_weight):
    f32 = np.float32
    fp8 = ml_dtypes.float8_e4m3
    emb_b = np.ascontiguousarray(np.asarray(emb, f32)).astype(bf16)

    def gmajor8(wT, ncols):
        # [E, ncols*128] -> [ncols, EC//2, 128, 2, 128] fp8 (DoubleRow pairs)
        a = wT.reshape(EC // 2, 2, 128, ncols, 128).transpose(3, 0, 2, 1, 4)
        return np.ascontiguousarray(np.clip(a * SW, -240, 240)).astype(fp8)
    wiT = gmajor8(np.ascontiguousarray(np.asarray(gru_Wi, f32).T), 3 * EC)
    whT = gmajor8(np.ascontiguousarray(np.asarray(gru_Wh, f32).T), 3 * EC)
    ws = gmajor8(np.ascontiguousarray(np.asarray(sent_weight, f32)), EC)
    ctxw = np.ascontiguousarray(np.asarray(context_weight, f32)).astype(bf16).reshape(EC, 128, 1)
    bi = np.asarray(gru_bi, f32)
    bh = np.asarray(gru_bh, f32)
    sb = np.asarray(sent_bias, f32).reshape(E)
    bias = np.zeros((128, 56), f32)
    for e in range(EC):
        bhn = bh[2 * E + e * 128:2 * E + (e + 1) * 128]
        bin_ = bi[2 * E + e * 128:2 * E + (e + 1) * 128]
        bias[:, e] = 0.5 * (bi + bh)[e * 128:(e + 1) * 128]
        bias[:, 8 + e] = 0.5 * (bi + bh)[E + e * 128:E + (e + 1) * 128]
        bias[:, 16 + e] = bin_
        bias[:, 24 + e] = SCL * bhn
        bias[:, 32 + e] = sb[e * 128:(e + 1) * 128]
        bias[:, 40 + e] = 0.5 * SCL * bhn
        bias[:, 48 + e] = bin_ + 0.5 * bhn
    return emb_b, wiT, whT, ws, ctxw, bias


def _core_tokens(tokens, core):
    """Build the [NB, 128] int32 gather-index blocks for one core."""
    tok = np.asarray(tokens)[core * T:(core + 1) * T].astype(np.int32)
    blocks = np.zeros((NB, 128), np.int32)
    for (lvl, c0, ncn, blist) in SCHEDULE:
        flat = tok[:, LEVEL_OFF[lvl]:LEVEL_OFF[lvl] + K**lvl].reshape(-1)
        for (gb, boff, rows) in blist:
            blocks[gb, :rows] = flat[c0 + boff:c0 + boff + rows]
    return blocks


def kernel(tokens, bs, emb, gru_Wi, gru_Wh, gru_bi, gru_bh,
           sent_weight, sent_bias, context_weight, _trace=False):
    from concourse import bass_utils
    bass_utils.upload_artifacts = lambda tmpdir: "local://" + tmpdir

    nc = get_nc()
    emb_b, wiT, whT, ws, ctxw, bias = _prep_shared(
        emb, gru_Wi, gru_Wh, gru_bi, gru_bh, sent_weight, sent_bias, context_weight)

    in_maps = []
    for c in range(NCORES):
        in_maps.append({
            "tok": _core_tokens(tokens, c),
            "emb": emb_b, "wiT": wiT, "whT": whT, "ws": ws, "ctxw": ctxw,
            "bias": bias,
        })
    res = bass_utils.run_bass_kernel_spmd(
        nc, in_maps, core_ids=list(range(NCORES)), trace=_trace)
    out = np.concatenate([res.results[c]["out"] for c in range(NCORES)], axis=0)
    if _trace:
        kernel.last_exec_time_ns = res.exec_time_ns
        kernel.last_results = res
    return out.astype(np.float32)



# revision 29
# speedup vs baseline: 1.9017x; 1.9017x over previous
"""Trainium2 Bass kernel for nn_BatchTreeEncoder (batched tree-GRU encoder).

Strategy
--------
Pure data parallel over the batch: 256 trees -> 32 trees on each of the 8
NeuronCores, weights replicated.  Inside one core all activations are kept in
a transposed [E, nodes] layout (E-chunks of 128 on partitions, nodes on the
free dim), so the only on-chip transposes are of the gathered embeddings.

Per level (bottom-up), in node-chunks, split into two pipelined phases:
  A(ci): indirect-DMA gather of embedding rows, PE-transpose to X^T, GRU
    gates as fp8 DoubleRow matmuls accumulated in f32 PSUM (weights x64,
    activations x16, the 1/1024 undone in the activation scale), then the
    GRU blend / running max / fp8 quantize as chunk-wide bulk DVE ops.
  B(ci): child attention for the parent level: U = tanh(Ws^T H) (fp8 DR),
    s = tanh(cw^T U), softmax broadcast to 128 partitions via a K=1 matmul
    and normalized as wide [128, n] ops, weighted child sum as 3 bulk ops.
A(ci+1) is emitted before B(ci) so the tensor engine always has the next
chunk's (independent) GX matmuls available while chunk ci's blend/quantize
chain drains on the vector/scalar engines.  sigmoid(x) = 0.5*(1+tanh(x/2))
keeps tanh/exp the only ACT tables.  The next chunk's transposes are paced
one-per-gate-group inside A's e-loop so the PSUM transpose bank ping-pong
hides under gate matmuls.
Output: PE-transpose of the [E, 32] running max back to [32, E], DMA out.
"""

import sys

for _p in ("/opt/trn_rl_repo",):
    if _p not in sys.path:
        sys.path.insert(0, _p)

import numpy as np
import ml_dtypes

bf16 = ml_dtypes.bfloat16

# ---------------------------------------------------------------- constants
NCORES = 8
BS = 256
T = BS // NCORES          # trees per core
K = 3
DEPTH = 4
E = 1024
EC = E // 128             # 8 e-chunks
VOCAB = 20000
N_NODES = sum(K**l for l in range(DEPTH + 1))   # 121
# fp8 quantization scales: weights x64, activations/emb x16 (keeps the bulk of
# the values out of e4m3's subnormal range); PSUM results carry x1024, undone
# in the activation `scale` params.
SW = 64.0
SX = 16.0
SCL = SW * SX
LEVEL_OFF = [sum(K**i for i in range(l)) for l in range(DEPTH + 1)]  # [0,1,4,13,40]

# node-chunk sizes per level; every chunk size is a multiple of 3^l (whole
# trees stay inside one chunk for the max-reduction) and of 3 for l>0 (whole
# sibling groups stay inside one chunk for the parent attention).
CHUNK_SIZES = {4: [486] * 5 + [162], 3: [432] * 2, 2: [288], 1: [96], 0: [32]}


def _schedule():
    """Static per-core schedule: list of (level, c0, nc, [(gblock, boff, rows)])."""
    sched = []
    gb = 0
    for l in range(DEPTH, -1, -1):
        c0 = 0
        for nc_ in CHUNK_SIZES[l]:
            blocks = []
            boff = 0
            while boff < nc_:
                rows = min(128, nc_ - boff)
                blocks.append((gb, boff, rows))
                gb += 1
                boff += rows
            sched.append((l, c0, nc_, blocks))
            c0 += nc_
    return sched, gb


SCHEDULE, NB = _schedule()
NCHUNK = len(SCHEDULE)


def _phase_order():
    """A/B phase order: A(ci+1) before B(ci), respecting level boundaries."""
    lvl = [SCHEDULE[ci][0] for ci in range(NCHUNK)]
    # children chunk range required by A(ci) (chunks of level lvl+1)
    def required_b(ci):
        l, c0, ncn, _ = SCHEDULE[ci]
        if l == DEPTH:
            return []
        lo, hi = 3 * c0, 3 * (c0 + ncn)
        req = []
        for cj in range(NCHUNK):
            lj, cj0, ncj, _ = SCHEDULE[cj]
            if lj == l + 1 and cj0 < hi and cj0 + ncj > lo:
                req.append(cj)
        return req

    order = []
    emitted_b = set()
    for ci in range(NCHUNK):
        for cj in required_b(ci):
            if cj not in emitted_b:
                order.append(("B", cj))
                emitted_b.add(cj)
        order.append(("A", ci))
        cj = ci - 1
        if cj >= 0 and lvl[cj] > 0 and cj not in emitted_b:
            order.append(("B", cj))
            emitted_b.add(cj)
    for cj in range(NCHUNK):
        if lvl[cj] > 0 and cj not in emitted_b:
            order.append(("B", cj))
    return order


PHASES = _phase_order()

_NC_CACHE = {}


# ---------------------------------------------------------------- builder
def build_nc():
    import concourse.bacc as bacc
    import concourse.bass as bass
    import concourse.mybir as mybir
    import concourse.tile as tile
    from concourse.masks import make_identity

    dt = mybir.dt
    Act = mybir.ActivationFunctionType
    Alu = mybir.AluOpType
    X = mybir.AxisListType.X

    nc = bacc.Bacc("TRN2", target_bir_lowering=False, debug=False)

    tok_d = nc.dram_tensor("tok", [NB, 128], dt.int32, kind="ExternalInput")
    emb_d = nc.dram_tensor("emb", [VOCAB, E], dt.bfloat16, kind="ExternalInput")
    wiT_d = nc.dram_tensor("wiT", [3 * EC, EC // 2, 128, 2, 128], dt.float8e4, kind="ExternalInput")
    whT_d = nc.dram_tensor("whT", [3 * EC, EC // 2, 128, 2, 128], dt.float8e4, kind="ExternalInput")
    ws_d = nc.dram_tensor("ws", [EC, EC // 2, 128, 2, 128], dt.float8e4, kind="ExternalInput")
    ctx_d = nc.dram_tensor("ctxw", [EC, 128, 1], dt.bfloat16, kind="ExternalInput")
    bias_d = nc.dram_tensor("bias", [128, 56], dt.float32, kind="ExternalInput")
    out_d = nc.dram_tensor("out", [T, E], dt.float32, kind="ExternalOutput")

    from contextlib import ExitStack

    with tile.TileContext(nc) as tc, ExitStack() as ctx:
        sing = ctx.enter_context(tc.tile_pool(name="sing", bufs=1))
        hsp = ctx.enter_context(tc.tile_pool(name="hsp", bufs=1))
        mp_ = ctx.enter_context(tc.tile_pool(name="mp", bufs=1))
        xrowp = ctx.enter_context(tc.tile_pool(name="xrowp", bufs=8))
        xtp = ctx.enter_context(tc.tile_pool(name="xtp", bufs=3))
        gp = ctx.enter_context(tc.tile_pool(name="gp", bufs=2))      # r/tt tiles
        bp = ctx.enter_context(tc.tile_pool(name="bp", bufs=1))      # zt/n/dd/mm
        hp = ctx.enter_context(tc.tile_pool(name="hp", bufs=2))      # H chunks
        up = ctx.enter_context(tc.tile_pool(name="up", bufs=2))      # U tiles
        rowp = ctx.enter_context(tc.tile_pool(name="rowp", bufs=1))  # softmax rows
        wp = ctx.enter_context(tc.tile_pool(name="wp", bufs=1))      # softmax/hsum
        psp = ctx.enter_context(tc.tile_pool(name="psp", bufs=1, space="PSUM"))

        # ---- persistent / constant tiles
        wiT = sing.tile([128, 3 * EC, EC // 2, 2, 128], dt.float8e4)
        whT = sing.tile([128, 3 * EC, EC // 2, 2, 128], dt.float8e4)
        ws = sing.tile([128, EC, EC // 2, 2, 128], dt.float8e4)
        ctxw = sing.tile([128, EC, 1], dt.bfloat16)
        biases = sing.tile([128, 56], dt.float32)
        identb = sing.tile([128, 128], dt.bfloat16)
        identf = sing.tile([128, 128], dt.float32)
        ones = sing.tile([1, 128], dt.bfloat16)
        idx = sing.tile([128, NB], dt.int32)

        # index + small tensors first (they gate the gather -> transpose chain),
        # then weights gate-chunk by gate-chunk in consumption order: Wi fully
        # before Wh (Wh is first needed only at level 3, ~250us in).
        nc.sync.dma_start(out=idx[:], in_=tok_d.rearrange("b p -> p b"))
        nc.sync.dma_start(out=biases[:], in_=bias_d[:])
        nc.sync.dma_start(out=ctxw[:, :, 0], in_=ctx_d.rearrange("k p o -> p (k o)"))
        make_identity(nc, identb[:])
        make_identity(nc, identf[:])
        nc.vector.memset(ones[:], 1.0)
        gorder = [base + e for e in range(EC) for base in (0, EC, 2 * EC)]
        for g in gorder:
            nc.sync.dma_start(out=wiT[:, g, :, :, :],
                              in_=wiT_d[g].rearrange("j p i m -> p j i m"))
        for g in range(EC):
            nc.sync.dma_start(out=ws[:, g, :, :, :],
                              in_=ws_d[g].rearrange("j p i m -> p j i m"))
        for g in gorder:
            nc.sync.dma_start(out=whT[:, g, :, :, :],
                              in_=whT_d[g].rearrange("j p i m -> p j i m"))

        # bias column helpers: cols 0..7 = 0.5*(bi+bh)_r, 8..15 = 0.5*(bi+bh)_z,
        # 16..23 = bi_n, 24..31 = SCL*bh_n, 32..39 = sent_bias,
        # 40..47 = 0.5*SCL*bh_n (leaf), 48..55 = bi_n + 0.5*bh_n (leaf)
        def bcol(c):
            return biases[:, c:c + 1]

        # running max, [128, EC, T] f32 (leaf level writes every slot first)
        msb = mp_.tile([128, EC, T], dt.float32)

        # per-level HS accumulation targets ([E, N_l] as [128, EC, N_l] bf16)
        hs_sb = {}
        hs8_sb = {}
        for l in range(DEPTH):
            n_l = T * K**l
            hs_sb[l] = hsp.tile([128, EC, n_l], dt.bfloat16, name=f"hs{l}")
            hs8_sb[l] = hsp.tile([128, EC, n_l], dt.float8e4, name=f"hs8{l}")

        DR = mybir.MatmulPerfMode.DoubleRow

        def gh_mms(out_ap, g, lvl, c0, ncn, start):
            src8 = hs8_sb[lvl]
            if ncn >= 128:
                for j in range(EC // 2):
                    nc.tensor.matmul(
                        out=out_ap, lhsT=whT[:, g, j, :, :],
                        rhs=src8[:, 2 * j:2 * j + 2, c0:c0 + ncn],
                        start=(start and j == 0), stop=(j == EC // 2 - 1),
                        perf_mode=DR)
            else:
                for k in range(EC):
                    nc.tensor.matmul(
                        out=out_ap, lhsT=whT[:, g, k // 2, k % 2, :],
                        rhs=src8[:, k, c0:c0 + ncn],
                        start=(start and k == 0), stop=(k == EC - 1))

        def gx_mms(out_ap, g, xt, ncn, stop):
            if ncn >= 128:
                for j in range(EC // 2):
                    nc.tensor.matmul(
                        out=out_ap, lhsT=wiT[:, g, j, :, :],
                        rhs=xt[:, 2 * j:2 * j + 2, :ncn],
                        start=(j == 0), stop=(stop and j == EC // 2 - 1),
                        perf_mode=DR)
            else:
                for k in range(EC):
                    nc.tensor.matmul(
                        out=out_ap, lhsT=wiT[:, g, k // 2, k % 2, :],
                        rhs=xt[:, k, :ncn],
                        start=(k == 0), stop=(stop and k == EC - 1))

        def u_mms(out_ap, f, h8, ncn):
            if ncn >= 128:
                for j in range(EC // 2):
                    nc.tensor.matmul(
                        out=out_ap, lhsT=ws[:, f, j, :, :],
                        rhs=h8[:, 2 * j:2 * j + 2, :],
                        start=(j == 0), stop=(j == EC // 2 - 1), perf_mode=DR)
            else:
                for k in range(EC):
                    nc.tensor.matmul(
                        out=out_ap, lhsT=ws[:, f, k // 2, k % 2, :],
                        rhs=h8[:, k, :],
                        start=(k == 0), stop=(k == EC - 1))

        # ---- gather / transpose plumbing (prefetched two phases ahead)
        pend = {}

        def emit_gather(ci):
            lvl, c0, ncn, blocks = SCHEDULE[ci]
            xrows = []
            for (gb, boff, rows) in blocks:
                xrow = xrowp.tile([128, E], dt.bfloat16, name="xrow")
                nc.gpsimd.indirect_dma_start(
                    out=xrow[:rows, :],
                    out_offset=None,
                    in_=emb_d[:, :],
                    in_offset=bass.IndirectOffsetOnAxis(ap=idx[:rows, gb:gb + 1], axis=0),
                )
                xrows.append((xrow, boff, rows))
            xt = xtp.tile([128, EC, ncn], dt.float8e4, name="xt")
            pend[ci] = (xrows, xt, ncn)

        def emit_xt_e(ci, e):
            xrows, xt, ncn = pend[ci]
            tp = psp.tile([128, 512], dt.bfloat16, name="tp", tag="tp", bufs=2)
            for (xrow, boff, rows) in xrows:
                nc.tensor.transpose(
                    out=tp[:, boff:boff + rows],
                    in_=xrow[:rows, e * 128:(e + 1) * 128],
                    identity=identb[:rows, :rows],
                )
            nc.vector.tensor_scalar_mul(out=xt[:, e, :], in0=tp[:, :ncn],
                                        scalar1=SX)

        hch_t = {}
        hch8_t = {}

        # ---------------- phase A: gather/gates/blend/max/quantize
        def emit_A(ci):
            lvl, c0, ncn, blocks = SCHEDULE[ci]
            leaf = lvl == DEPTH
            n_per_tree = K**lvl
            tr0 = c0 // n_per_tree
            ntr = ncn // n_per_tree
            if ci + 2 < NCHUNK:
                emit_gather(ci + 2)
            xt = pend[ci][1]

            hch = hp.tile([128, EC, ncn], dt.bfloat16, name="hch", tag="hch")
            hch_t[ci] = hch
            if lvl > 0:
                hch8_t[ci] = hp.tile([128, EC, ncn], dt.float8e4, name="hch8",
                                     tag="hch8")
            zt = bp.tile([128, EC, ncn], dt.bfloat16, name="zt", tag="zt")
            nt = bp.tile([128, EC, ncn], dt.bfloat16, name="nt", tag="nt")
            for e in range(EC):
                if ci + 1 < NCHUNK:
                    emit_xt_e(ci + 1, e)
                # r gate
                psr = psp.tile([128, 512], dt.float32, name="psr", tag="acc", bufs=2)
                gx_mms(psr[:, :ncn], e, xt, ncn, stop=leaf)
                if not leaf:
                    gh_mms(psr[:, :ncn], e, lvl, c0, ncn, start=False)
                # r tile holds rt = tanh(rin/2); non-leaf converts to
                # r = 0.5 + 0.5*rt, the leaf path folds that into the n-gate
                # bias columns instead.
                r = gp.tile([128, 512], dt.bfloat16, name="r", tag="r")
                nc.scalar.activation(r[:, :ncn], psr[:, :ncn], Act.Tanh,
                                     bias=bcol(e), scale=0.5 / SCL)
                if not leaf:
                    nc.vector.tensor_scalar(r[:, :ncn], r[:, :ncn], 0.5, 0.5,
                                            Alu.mult, Alu.add)
                # z gate (kept as zt = tanh(zin/2))
                psz = psp.tile([128, 512], dt.float32, name="psz", tag="acc", bufs=2)
                gx_mms(psz[:, :ncn], EC + e, xt, ncn, stop=leaf)
                if not leaf:
                    gh_mms(psz[:, :ncn], EC + e, lvl, c0, ncn, start=False)
                nc.scalar.activation(zt[:, e, :], psz[:, :ncn], Act.Tanh,
                                     bias=bcol(8 + e), scale=0.5 / SCL)
                # n gate
                psx = psp.tile([128, 512], dt.float32, name="psx", tag="gxn", bufs=1)
                gx_mms(psx[:, :ncn], 2 * EC + e, xt, ncn, stop=True)
                tt = gp.tile([128, 512], dt.bfloat16, name="tt", tag="tt")
                if leaf:
                    # tt = rt * 0.5*SCL*bh_n + GXn (the remaining 0.5*bh_n
                    # constant lives in the leaf n-gate bias column)
                    nc.vector.scalar_tensor_tensor(
                        out=tt[:, :ncn], in0=r[:, :ncn], scalar=bcol(40 + e),
                        in1=psx[:, :ncn], op0=Alu.mult, op1=Alu.add)
                else:
                    psh = psp.tile([128, 512], dt.float32, name="psh", tag="ghn", bufs=2)
                    gh_mms(psh[:, :ncn], 2 * EC + e, lvl, c0, ncn, start=True)
                    # tt = (GHn + SCL*bh_n) * r ; then += GXn
                    nc.vector.scalar_tensor_tensor(
                        out=tt[:, :ncn], in0=psh[:, :ncn], scalar=bcol(24 + e),
                        in1=r[:, :ncn], op0=Alu.add, op1=Alu.mult)
                    nc.vector.tensor_add(tt[:, :ncn], tt[:, :ncn], psx[:, :ncn])
                nc.scalar.activation(nt[:, e, :], tt[:, :ncn], Act.Tanh,
                                     bias=bcol((48 if leaf else 16) + e),
                                     scale=1.0 / SCL)

            # ---- chunk-wide bulk blend -> H
            if leaf:
                nc.vector.tensor_scalar(zt[:, :, :], zt[:, :, :], -0.5, 0.5,
                                        Alu.mult, Alu.add)
                nc.vector.tensor_mul(hch[:, :, :], zt[:, :, :], nt[:, :, :])
            else:
                hsv = hs_sb[lvl][:, :, c0:c0 + ncn]
                dd = bp.tile([128, EC, ncn], dt.bfloat16, name="dd", tag="dd")
                nc.vector.tensor_sub(dd[:, :, :], hsv, nt[:, :, :])
                mm_ = bp.tile([128, EC, ncn], dt.bfloat16, name="mm_", tag="mm_")
                nc.vector.tensor_mul(mm_[:, :, :], zt[:, :, :], dd[:, :, :])
                nc.vector.tensor_add(mm_[:, :, :], dd[:, :, :], mm_[:, :, :])
                nc.vector.scalar_tensor_tensor(
                    out=hch[:, :, :], in0=mm_[:, :, :], scalar=0.5,
                    in1=nt[:, :, :], op0=Alu.mult, op1=Alu.add)
            # ---- bulk running max
            msv = msb[:, :, tr0:tr0 + ntr]
            if leaf:
                nc.vector.reduce_max(
                    out=msv,
                    in_=hch[:, :, :].rearrange("p e (t q) -> p e t q",
                                               q=n_per_tree),
                    axis=X)
            elif n_per_tree == 1:
                nc.vector.tensor_max(msv, msv, hch[:, :, :])
            else:
                red = wp.tile([128, EC, T], dt.float32, name="red", tag="red")
                nc.vector.reduce_max(
                    out=red[:, :, :ntr],
                    in_=hch[:, :, :].rearrange("p e (t q) -> p e t q",
                                               q=n_per_tree),
                    axis=X)
                nc.vector.tensor_max(msv, msv, red[:, :, :ntr])
            # ---- bulk fp8 quantize of H for the attention matmuls
            if lvl > 0:
                nc.scalar.mul(out=hch8_t[ci][:, :, :], in_=hch[:, :, :], mul=SX)

        # ---------------- phase B: child attention -> parent hsum
        def emit_B(ci):
            lvl, c0, ncn, blocks = SCHEDULE[ci]
            npar = ncn // 3
            p0 = c0 // 3
            lp = lvl - 1
            hch = hch_t.pop(ci)
            hch8 = hch8_t.pop(ci)
            pss = psp.tile([1, 512], dt.float32, name="pss", tag="srow", bufs=1)
            for f in range(EC):
                psu = psp.tile([128, 512], dt.float32, name="psu", tag="acc", bufs=2)
                u_mms(psu[:, :ncn], f, hch8, ncn)
                ut = up.tile([128, 512], dt.bfloat16, name="ut", tag="ut")
                nc.scalar.activation(ut[:, :ncn], psu[:, :ncn], Act.Tanh,
                                     bias=bcol(32 + f), scale=1.0 / SCL)
                nc.tensor.matmul(out=pss[:, :ncn], lhsT=ctxw[:, f, 0:1],
                                 rhs=ut[:, :ncn],
                                 start=(f == 0), stop=(f == EC - 1))
            # softmax: tanh -> broadcast to 128 partitions via ones-matmul
            # -> wide exp / 3-way normalize (no 1-partition DVE ops)
            srow = rowp.tile([1, 512], dt.bfloat16, name="srow", tag="srow")
            nc.scalar.activation(srow[:, :ncn], pss[:, :ncn], Act.Tanh)
            psb = psp.tile([128, 512], dt.float32, name="psb", tag="ghn", bufs=2)
            nc.tensor.matmul(out=psb[:, :ncn], lhsT=ones[:, :],
                             rhs=srow[:, :ncn], start=True, stop=True)
            ebc = wp.tile([128, 512], dt.bfloat16, name="ebc", tag="ebc")
            nc.scalar.activation(ebc[:, :ncn], psb[:, :ncn], Act.Exp)
            e3 = ebc[:, :ncn].rearrange("p (n k) -> p n k", k=3)
            den = wp.tile([128, 170], dt.float32, name="den", tag="den")
            nc.vector.tensor_add(den[:, :npar], e3[:, :, 0], e3[:, :, 1])
            nc.vector.tensor_add(den[:, :npar], den[:, :npar], e3[:, :, 2])
            rinv = wp.tile([128, 170], dt.float32, name="rinv", tag="rinv")
            nc.vector.reciprocal(rinv[:, :npar], den[:, :npar])
            wb = wp.tile([128, 512], dt.bfloat16, name="wb", tag="wb")
            w3 = wb[:, :ncn].rearrange("p (n k) -> p n k", k=3)
            for kk in range(3):
                nc.vector.tensor_mul(w3[:, :, kk], e3[:, :, kk], rinv[:, :npar])
            # bulk weighted child sum -> hsum for the parent level
            pp = wp.tile([128, EC, ncn], dt.bfloat16, name="pp", tag="pp")
            nc.vector.tensor_mul(
                pp[:, :, :], hch[:, :, :],
                wb[:, None, :ncn].broadcast_to([128, EC, ncn]))
            pp4 = pp[:, :, :].rearrange("p e (q k) -> p e q k", k=3)
            ta = wp.tile([128, EC, 170], dt.bfloat16, name="ta", tag="ta")
            nc.vector.tensor_add(ta[:, :, :npar], pp4[:, :, :, 0], pp4[:, :, :, 1])
            nc.vector.tensor_add(hs_sb[lp][:, :, p0:p0 + npar],
                                 ta[:, :, :npar], pp4[:, :, :, 2])
            nc.scalar.mul(out=hs8_sb[lp][:, :, p0:p0 + npar],
                          in_=hs_sb[lp][:, :, p0:p0 + npar], mul=SX)

        # ---------------- emit all phases
        emit_gather(0)
        for e in range(EC):
            emit_xt_e(0, e)
        emit_gather(1)
        for kind, ci in PHASES:
            if kind == "A":
                emit_A(ci)
            else:
                emit_B(ci)

        # ---------------- output: transpose msb -> [T, E], DMA out
        osb = mp_.tile([T, E], dt.float32)
        for e in range(EC):
            po = psp.tile([T, 128], dt.float32, name="po", tag="tp", bufs=2)
            nc.tensor.transpose(out=po[:, :], in_=msb[:, e, :], identity=identf[:, :])
            nc.vector.tensor_copy(out=osb[:, e * 128:(e + 1) * 128], in_=po[:, :])
        nc.sync.dma_start(out=out_d[:, :], in_=osb[:, :])

    nc.compile()
    return nc


def get_nc():
    if "nc" not in _NC_CACHE:
        _NC_CACHE["nc"] = build_nc()
    return _NC_CACHE["nc"]


# ---------------------------------------------------------------- host side
def _prep_shared(emb, gru_Wi, gru_Wh, gru_bi, gru_bh, sent_weight, sent_bias,
                 context_weight):
    f32 = np.float32
    fp8 = ml_dtypes.float8_e4m3
    emb_b = np.ascontiguousarray(np.asarray(emb, f32)).astype(bf16)

    def gmajor8(wT, ncols):
        # [E, ncols*128] -> [ncols, EC//2, 128, 2, 128] fp8 (DoubleRow pairs)
        a = wT.reshape(EC // 2, 2, 128, ncols, 128).transpose(3, 0, 2, 1, 4)
        return np.ascontiguousarray(np.clip(a * SW, -240, 240)).astype(fp8)
    wiT = gmajor8(np.ascontiguousarray(np.asarray(gru_Wi, f32).T), 3 * EC)
    whT = gmajor8(np.ascontiguousarray(np.asarray(gru_Wh, f32).T), 3 * EC)
    ws = gmajor8(np.ascontiguousarray(np.asarray(sent_weight, f32)), EC)
    ctxw = np.ascontiguousarray(np.asarray(context_weight, f32)).astype(bf16).reshape(EC, 128, 1)
    bi = np.asarray(gru_bi, f32)
    bh = np.asarray(gru_bh, f32)
    sb = np.asarray(sent_bias, f32).reshape(E)
    bias = np.zeros((128, 56), f32)
    for e in range(EC):
        bhn = bh[2 * E + e * 128:2 * E + (e + 1) * 128]
        bin_ = bi[2 * E + e * 128:2 * E + (e + 1) * 128]
        bias[:, e] = 0.5 * (bi + bh)[e * 128:(e + 1) * 128]
        bias[:, 8 + e] = 0.5 * (bi + bh)[E + e * 128:E + (e + 1) * 128]
        bias[:, 16 + e] = bin_
        bias[:, 24 + e] = SCL * bhn
        bias[:, 32 + e] = sb[e * 128:(e + 1) * 128]
        bias[:, 40 + e] = 0.5 * SCL * bhn
        bias[:, 48 + e] = bin_ + 0.5 * bhn
    return emb_b, wiT, whT, ws, ctxw, bias


def _core_tokens(tokens, core):
    """Build the [NB, 128] int32 gather-index blocks for one core."""
    tok = np.asarray(tokens)[core * T:(core + 1) * T].astype(np.int32)
    blocks = np.zeros((NB, 128), np.int32)
    for (lvl, c0, ncn, blist) in SCHEDULE:
        flat = tok[:, LEVEL_OFF[lvl]:LEVEL_OFF[lvl] + K**lvl].reshape(-1)
        for (gb, boff, rows) in blist:
            blocks[gb, :rows] = flat[c0 + boff:c0 + boff + rows]
    return blocks


def kernel(tokens, bs, emb, gru_Wi, gru_Wh, gru_bi, gru_bh,
           sent_weight, sent_bias, context_weight, _trace=False):
    from concourse import bass_utils
    bass_utils.upload_artifacts = lambda tmpdir: "local://" + tmpdir

    nc = get_nc()
    emb_b, wiT, whT, ws, ctxw, bias = _prep_shared(
        emb, gru_Wi, gru_Wh, gru_bi, gru_bh, sent_weight, sent_bias, context_weight)

    in_maps = []
    for c in range(NCORES):
        in_maps.append({
            "tok": _core_tokens(tokens, c),
            "emb": emb_b, "wiT": wiT, "whT": whT, "ws": ws, "ctxw": ctxw,
            "bias": bias,
        })
    res = bass_utils.run_bass_kernel_spmd(
        nc, in_maps, core_ids=list(range(NCORES)), trace=_trace)
    out = np.concatenate([res.results[c]["out"] for c in range(NCORES)], axis=0)
    if _trace:
        kernel.last_exec_time_ns = res.exec_time_ns
        kernel.last_results = res
    return out.astype(np.float32)
